# revision 6
# baseline (speedup 1.0000x reference)
"""Trainium2 Bass kernel for nn_MixtureOfLayersClassifier.

Data-parallel over 8 NeuronCores: each core processes 32 of the 256 batch
rows end-to-end (pooling -> gate MLP -> per-stage projections with
LayerNorm+GELU -> top-2 routing -> mixture -> classifier MLP). No
collectives; the host concatenates per-core outputs.

Self-contained: hardcodes all shapes from the problem spec.
"""

import numpy as np

import concourse.bacc as bacc
import concourse.bass as bass
import concourse.tile as tile
from concourse import mybir
from concourse.bass_utils import run_bass_kernel_spmd
from concourse.masks import make_identity

F32 = mybir.dt.float32
AF = mybir.ActivationFunctionType
ALU = mybir.AluOpType
AX = mybir.AxisListType

NCORES = 8
B = 256
BL = B // NCORES  # 32 rows per core
F = 512
H = 1024
C = 1000
LN_EPS = 1e-5

# name, Cin, spatial, reduce-chunking (n_dma, rows_per_dma_col, spatial_split)
STAGES = {
    "s1": dict(cin=16, hw=112),
    "s2": dict(cin=24, hw=56),
    "s3": dict(cin=40, hw=28),
    "s4": dict(cin=80, hw=14),
    "s5": dict(cin=160, hw=7),
    "ff": dict(cin=960, hw=7),
}

LAST_RESULTS = None  # set by kernel(); test harness reads trace info from here


def _build():
    nc = bacc.Bacc("TRN2", target_bir_lowering=False, debug=False,
                   num_devices=NCORES)

    # ---- DRAM I/O (per-core shard shapes) ----
    d_in = {}
    for n, m in STAGES.items():
        d_in[n] = nc.dram_tensor(n, [BL, m["cin"], m["hw"], m["hw"]], F32,
                                 kind="ExternalInput")
    for i, cin in enumerate([16, 24, 40, 80, 160], start=1):
        d_in[f"Wp{i}"] = nc.dram_tensor(f"Wp{i}", [cin, F], F32, kind="ExternalInput")
        for v in (f"bp{i}", f"g{i}", f"be{i}"):
            d_in[v] = nc.dram_tensor(v, [F], F32, kind="ExternalInput")
    d_in["Wg1"] = nc.dram_tensor("Wg1", [960, H], F32, kind="ExternalInput")
    d_in["bg1"] = nc.dram_tensor("bg1", [H], F32, kind="ExternalInput")
    d_in["Wg2"] = nc.dram_tensor("Wg2", [H, H // 2], F32, kind="ExternalInput")
    d_in["bg2"] = nc.dram_tensor("bg2", [H // 2], F32, kind="ExternalInput")
    d_in["Wg3"] = nc.dram_tensor("Wg3", [H // 2, 5], F32, kind="ExternalInput")
    d_in["bg3"] = nc.dram_tensor("bg3", [5], F32, kind="ExternalInput")
    d_in["Wc1"] = nc.dram_tensor("Wc1", [F, H], F32, kind="ExternalInput")
    d_in["bc1"] = nc.dram_tensor("bc1", [H], F32, kind="ExternalInput")
    d_in["Wc2"] = nc.dram_tensor("Wc2", [H, H // 2], F32, kind="ExternalInput")
    d_in["bc2"] = nc.dram_tensor("bc2", [H // 2], F32, kind="ExternalInput")
    d_in["Wc3"] = nc.dram_tensor("Wc3", [H // 2, C], F32, kind="ExternalInput")
    d_in["bc3"] = nc.dram_tensor("bc3", [C], F32, kind="ExternalInput")

    out_logits = nc.dram_tensor("out_logits", [BL, C], F32, kind="ExternalOutput")
    out_gate = nc.dram_tensor("out_gate", [BL, 5], F32, kind="ExternalOutput")
    out_topi = nc.dram_tensor("out_topi", [BL, 2], mybir.dt.int32,
                              kind="ExternalOutput")
    out_w = nc.dram_tensor("out_w", [BL, 2], F32, kind="ExternalOutput")

    with tile.TileContext(nc) as tc:
        with (
            tc.tile_pool(name="consts", bufs=1) as consts,
            tc.tile_pool(name="wts", bufs=1) as wts,
            tc.tile_pool(name="stream", bufs=3) as stream,
            tc.tile_pool(name="trash", bufs=1) as trashp,
            tc.tile_pool(name="small", bufs=1) as small,
            tc.tile_pool(name="ln", bufs=3) as lnp,
            tc.tile_pool(name="stage", bufs=1) as stage,
            tc.tile_pool(name="hbuf", bufs=2) as hbuf,
            tc.tile_pool(name="psmm", bufs=4, space="PSUM") as psmm,
            tc.tile_pool(name="pstp", bufs=2, space="PSUM") as pstp,
            tc.tile_pool(name="dram", bufs=1, space="DRAM") as dramp,
        ):
            # ---------- constants ----------
            ident = consts.tile([128, 128], F32, tag="ident")
            make_identity(nc, ident)
            ones_col = consts.tile([128, 1], F32, tag="ones_col")
            nc.vector.memset(ones_col, 1.0)
            ones_row = consts.tile([1, 128], F32, tag="ones_row")
            nc.vector.memset(ones_row, 1.0)
            eps_t = consts.tile([1, 1], F32, tag="eps")
            nc.vector.memset(eps_t, LN_EPS)
            iota_i = consts.tile([BL, 5], mybir.dt.int32, tag="iota_i")
            nc.gpsimd.iota(iota_i, pattern=[[1, 5]], base=0, channel_multiplier=0)
            iota_f = consts.tile([BL, 5], F32, tag="iota_f")
            nc.vector.tensor_copy(iota_f, iota_i)

            trash = trashp.tile([128, 3136], F32, tag="trash")

            # ---------- pooling: stream + reduce one stage ----------
            def load_vec(name, p, k):
                """[N]-vector -> SBUF [p, k] with element (r, c) = v[c*p + r]."""
                t = wts.tile([p, k], F32, tag=f"v_{name}")
                nc.gpsimd.dma_start(
                    out=t, in_=d_in[name].ap().rearrange("(c p) -> p c", p=p))
                return t

            def pool_stage(name, use_act=False):
                """Mean over spatial dims -> poolT [(chunks of) Cin, BL] in SBUF."""
                m = STAGES[name]
                cin, s = m["cin"], m["hw"] * m["hw"]
                rows = BL * cin          # (b, c) row count
                cpp = rows // 128        # rows per partition
                flat = d_in[name].ap().rearrange("b c h w -> (b c) (h w)")
                view = flat.rearrange("(p r) s -> p r s", r=cpp)

                pcols = small.tile([128, cpp], F32, tag=f"pcols_{name}")
                if name == "s1":
                    # 16 DMAs of [128, 3136]; ScalarE accum-reduce; combine.
                    parts = small.tile([128, 4, 4], F32, tag="parts_s1")
                    for qr in range(4):
                        for h in range(4):
                            t = stream.tile([128, 3136], F32, tag="st")
                            nc.sync.dma_start(
                                out=t, in_=view[:, qr, h * 3136:(h + 1) * 3136])
                            nc.scalar.activation(
                                out=trash[:, :], in_=t, func=AF.Copy,
                                accum_out=parts[:, qr, h:h + 1])
                    nc.vector.tensor_reduce(out=pcols, in_=parts, axis=AX.X,
                                            op=ALU.add)
                elif name == "s2":
                    for q in range(6):
                        t = stream.tile([128, 3136], F32, tag="st")
                        nc.sync.dma_start(out=t, in_=view[:, q, :])
                        nc.vector.tensor_reduce(out=pcols[:, q:q + 1], in_=t,
                                                axis=AX.X, op=ALU.add)
                elif name == "s3":
                    for q in range(2):
                        t = stream.tile([128, 5, 784], F32, tag="st")
                        nc.sync.dma_start(out=t, in_=view[:, 5 * q:5 * q + 5, :])
                        nc.vector.tensor_reduce(out=pcols[:, 5 * q:5 * q + 5],
                                                in_=t, axis=AX.X, op=ALU.add)
                elif name == "s4":
                    t = stream.tile([128, 20, 196], F32, tag="st")
                    nc.sync.dma_start(out=t, in_=view[:, :, :])
                    nc.vector.tensor_reduce(out=pcols, in_=t, axis=AX.X, op=ALU.add)
                elif name == "s5":
                    t = stream.tile([128, 40, 49], F32, tag="st")
                    nc.sync.dma_start(out=t, in_=view[:, :, :])
                    nc.vector.tensor_reduce(out=pcols, in_=t, axis=AX.X, op=ALU.add)
                else:  # ff
                    for q in range(4):
                        t = stream.tile([128, 60, 49], F32, tag="st")
                        nc.sync.dma_start(out=t, in_=view[:, 60 * q:60 * q + 60, :])
                        nc.vector.tensor_reduce(out=pcols[:, 60 * q:60 * q + 60],
                                                in_=t, axis=AX.X, op=ALU.add)

                scaled = small.tile([128, cpp], F32, tag=f"psc_{name}")
                nc.scalar.mul(scaled, pcols, 1.0 / s)

                # bounce through DRAM to regroup rows -> [BL, cin]
                scr = dramp.tile([rows], F32, tag=f"scr_{name}")
                nc.gpsimd.dma_start(
                    out=scr.rearrange("(p r) -> p r", r=cpp), in_=scaled)
                bc = small.tile([BL, cin], F32, tag=f"bc_{name}")
                nc.gpsimd.dma_start(out=bc,
                                    in_=scr.rearrange("(b c) -> b c", c=cin))

                # PE-transpose chunks of <=128 -> poolT [cw, nk, BL]
                if cin <= 128:
                    chunks = [(0, cin)]
                else:
                    cw = cin // 2 if cin == 160 else 120
                    chunks = [(j, cw) for j in range(0, cin, cw)]
                nk = len(chunks)
                cw = chunks[0][1]
                pt = stage.tile([cw, nk, BL], F32, tag=f"pt_{name}")
                for ki, (c0, w) in enumerate(chunks):
                    pst = pstp.tile([128, 128], F32, tag="tp")
                    nc.tensor.transpose(pst[:w, 0:BL], bc[:, c0:c0 + w],
                                        ident[0:BL, 0:BL])
                    nc.scalar.copy(pt[:w, ki, :], pst[:w, 0:BL])
                return pt

            # ---------- generic dense layer in transposed orientation ----------
            def dense(w_sb, k_chunks, rhs_fn, n_out, bias_sb, act, out_sb,
                      out_col=None, n_part=128):
                """out[f_chunk] = act(sum_k w[k,f].T @ rhs[k] + bias).

                w_sb: SBUF weight tile [kp, nk, N_total]; rhs_fn(kc) -> [kp, BL];
                n_out: number of n_part-wide output chunks; out_sb [n_part, n_out, BL].
                """
                nk = w_sb.shape[1]
                for fc in range(n_out):
                    ps = psmm.tile([n_part, BL], F32, tag="mm")
                    for kc in range(nk):
                        nc.tensor.matmul(
                            ps,
                            w_sb[:, kc, fc * n_part:(fc + 1) * n_part],
                            rhs_fn(kc),
                            start=(kc == 0), stop=(kc == nk - 1))
                    col = fc if out_col is None else out_col
                    if act == "relu":
                        nc.scalar.activation(
                            out=out_sb[:, col, :], in_=ps, func=AF.Relu,
                            bias=bias_sb[:, fc:fc + 1], scale=1.0)
                    else:  # bias add only
                        nc.vector.tensor_scalar_add(
                            out_sb[:, col, :], ps, bias_sb[:, fc:fc + 1])

            # ---------- ff -> gate path ----------
            pt_ff = pool_stage("ff")

            wg1 = wts.tile([120, 8, H], F32, tag="wg1")
            nc.scalar.dma_start(
                out=wg1, in_=d_in["Wg1"].ap().rearrange("(k p) n -> p k n", p=120))
            wg2 = wts.tile([128, 8, H // 2], F32, tag="wg2")
            nc.scalar.dma_start(
                out=wg2, in_=d_in["Wg2"].ap().rearrange("(k p) n -> p k n", p=128))
            wg3 = wts.tile([128, 4, 5], F32, tag="wg3")
            nc.scalar.dma_start(
                out=wg3, in_=d_in["Wg3"].ap().rearrange("(k p) n -> p k n", p=128))
            bg1 = load_vec("bg1", 128, 8)
            bg2 = load_vec("bg2", 128, 4)
            bg3 = load_vec("bg3", 5, 1)

            g1 = hbuf.tile([128, 8, BL], F32, tag="g1")
            dense(wg1, 8, lambda kc: pt_ff[:, kc, :], 8, bg1, "relu", g1)
            g2 = hbuf.tile([128, 4, BL], F32, tag="g2")
            dense(wg2, 8, lambda kc: g1[:, kc, :], 4, bg2, "relu", g2)
            # gate logits [5, BL]
            ps_gl = psmm.tile([5, BL], F32, tag="mm")
            for kc in range(4):
                nc.tensor.matmul(ps_gl, wg3[:, kc, :], g2[:, kc, :],
                                 start=(kc == 0), stop=(kc == 3))
            gateT = small.tile([5, BL], F32, tag="gateT")
            nc.vector.tensor_scalar_add(gateT, ps_gl, bg3[:, 0:1])

            # ---------- routing ----------
            ps_g32 = pstp.tile([128, 128], F32, tag="tp")
            nc.tensor.transpose(ps_g32[0:BL, 0:5], gateT, ident[0:5, 0:5])
            gate8 = small.tile([BL, 8], F32, tag="gate8")
            nc.vector.memset(gate8, -1e30)
            nc.vector.tensor_copy(gate8[:, 0:5], ps_g32[0:BL, 0:5])
            nc.gpsimd.dma_start(out=out_gate[:], in_=gate8[:, 0:5])

            vmax = small.tile([BL, 8], F32, tag="vmax")
            vidx = small.tile([BL, 8], mybir.dt.uint32, tag="vidx")
            nc.vector.max_with_indices(vmax, vidx, gate8)

            topi_i = small.tile([BL, 2], mybir.dt.int32, tag="topi_i")
            nc.vector.tensor_copy(topi_i, vidx[:, 0:2])
            nc.gpsimd.dma_start(out=out_topi[:], in_=topi_i)

            dvt = small.tile([BL, 1], F32, tag="dvt")
            nc.vector.tensor_tensor(dvt, vmax[:, 1:2], vmax[:, 0:1], op=ALU.subtract)
            evt = small.tile([BL, 1], F32, tag="evt")
            nc.scalar.activation(out=evt, in_=dvt, func=AF.Exp)
            ope = small.tile([BL, 1], F32, tag="ope")
            nc.vector.tensor_scalar_add(ope, evt, 1.0)
            wpair = small.tile([BL, 2], F32, tag="wpair")
            nc.vector.reciprocal(wpair[:, 0:1], ope)          # w0 = 1/(1+e)
            nc.vector.tensor_scalar(wpair[:, 1:2], wpair[:, 0:1], 1.0, -1.0,
                                    op0=ALU.subtract, op1=ALU.mult)  # w1 = 1-w0
            nc.gpsimd.dma_start(out=out_w[:], in_=wpair)

            topi_f = small.tile([BL, 2], F32, tag="topi_f")
            nc.vector.tensor_copy(topi_f, vidx[:, 0:2])
            m0 = small.tile([BL, 5], F32, tag="m0")
            nc.vector.tensor_scalar(m0, iota_f, topi_f[:, 0:1], wpair[:, 0:1],
                                    op0=ALU.is_equal, op1=ALU.mult)
            m1 = small.tile([BL, 5], F32, tag="m1")
            nc.vector.tensor_scalar(m1, iota_f, topi_f[:, 1:2], wpair[:, 1:2],
                                    op0=ALU.is_equal, op1=ALU.mult)
            msel = small.tile([BL, 5], F32, tag="msel")
            nc.vector.tensor_tensor(msel, m0, m1, op=ALU.add)

            # broadcast m to all partitions: bounce + 0-step partition DMA
            scr_m = dramp.tile([BL * 5], F32, tag="scr_m")
            nc.gpsimd.dma_start(out=scr_m.rearrange("(b i) -> b i", i=5), in_=msel)
            bcast_m = small.tile([128, BL, 5], F32, tag="bcast_m")
            nc.gpsimd.dma_start(
                out=bcast_m,
                in_=bass.AP(tensor=scr_m.tensor, offset=scr_m.offset,
                            ap=[[0, 128], [5, BL], [1, 5]]))

            # ---------- remaining weights ----------
            wp = {}
            for i, cin in enumerate([16, 24, 40, 80], start=1):
                wp[i] = wts.tile([cin, 1, F], F32, tag=f"wp{i}", name=f"wp{i}")
                nc.scalar.dma_start(
                    out=wp[i],
                    in_=d_in[f"Wp{i}"].ap().rearrange("k (u n) -> k u n", u=1))
            wp[5] = wts.tile([80, 2, F], F32, tag="wp5", name="wp5")
            nc.scalar.dma_start(
                out=wp[5], in_=d_in["Wp5"].ap().rearrange("(k p) n -> p k n", p=80))
            bp = {i: load_vec(f"bp{i}", 128, 4) for i in range(1, 6)}
            gsc = {i: load_vec(f"g{i}", 128, 4) for i in range(1, 6)}
            bes = {i: load_vec(f"be{i}", 128, 4) for i in range(1, 6)}

            wc1 = wts.tile([128, 4, H], F32, tag="wc1")
            nc.scalar.dma_start(
                out=wc1, in_=d_in["Wc1"].ap().rearrange("(k p) n -> p k n", p=128))
            wc2 = wts.tile([128, 8, H // 2], F32, tag="wc2")
            nc.scalar.dma_start(
                out=wc2, in_=d_in["Wc2"].ap().rearrange("(k p) n -> p k n", p=128))
            wc3 = wts.tile([128, 4, C], F32, tag="wc3")
            nc.scalar.dma_start(
                out=wc3, in_=d_in["Wc3"].ap().rearrange("(k p) n -> p k n", p=128))
            bc1 = load_vec("bc1", 128, 8)
            bc2 = load_vec("bc2", 128, 4)
            bc3 = load_vec("bc3", 125, 8)

            # ---------- per-stage projections -> stack ----------
            stack = [stage.tile([128, BL, 5], F32, tag=f"stack{c}",
                                name=f"stack{c}")
                     for c in range(4)]

            def project(i, pt):
                h_sb = hbuf.tile([128, 4, BL], F32, tag="h_sb")
                nk = pt.shape[1]
                for fc in range(4):
                    ps = psmm.tile([128, BL], F32, tag="mm")
                    for kc in range(nk):
                        nc.tensor.matmul(
                            ps, wp[i][:, kc, fc * 128:(fc + 1) * 128],
                            pt[:, kc, :],
                            start=(kc == 0), stop=(kc == nk - 1))
                    nc.vector.tensor_scalar_add(h_sb[:, fc, :], ps,
                                                bp[i][:, fc:fc + 1])
                # LayerNorm stats over F (partition axis) via ones-matmul
                hsq = lnp.tile([128, 4, BL], F32, tag="hsq")
                nc.vector.tensor_tensor(hsq, h_sb, h_sb, op=ALU.mult)
                ps_s = psmm.tile([1, BL], F32, tag="mm")
                for c in range(4):
                    nc.tensor.matmul(ps_s, ones_col, h_sb[:, c, :],
                                     start=(c == 0), stop=(c == 3))
                ps_q = psmm.tile([1, BL], F32, tag="mm")
                for c in range(4):
                    nc.tensor.matmul(ps_q, ones_col, hsq[:, c, :],
                                     start=(c == 0), stop=(c == 3))
                srow = lnp.tile([1, 2 * BL], F32, tag="srow")
                nc.vector.tensor_scalar_mul(srow[:, 0:BL], ps_s, 1.0 / F)  # mu
                ms = lnp.tile([1, BL], F32, tag="ms")
                nc.vector.tensor_scalar_mul(ms, ps_q, 1.0 / F)             # E[h^2]
                musq = lnp.tile([1, BL], F32, tag="musq")
                nc.vector.tensor_tensor(musq, srow[:, 0:BL], srow[:, 0:BL],
                                        op=ALU.mult)
                var = lnp.tile([1, BL], F32, tag="var")
                nc.vector.tensor_tensor(var, ms, musq, op=ALU.subtract)
                sd = lnp.tile([1, BL], F32, tag="sd")
                nc.scalar.activation(out=sd, in_=var, func=AF.Sqrt,
                                     bias=eps_t[:, 0:1], scale=1.0)
                nc.vector.reciprocal(srow[:, BL:2 * BL], sd)               # rstd
                ps_bc = psmm.tile([128, 2 * BL], F32, tag="mm")
                nc.tensor.matmul(ps_bc, ones_row, srow, start=True, stop=True)
                bcst = lnp.tile([128, 2 * BL], F32, tag="bcst")
                nc.scalar.copy(bcst, ps_bc)
                for fc in range(4):
                    cen = lnp.tile([128, BL], F32, tag="cen")
                    nc.vector.tensor_tensor(cen, h_sb[:, fc, :], bcst[:, 0:BL],
                                            op=ALU.subtract)
                    nrm = lnp.tile([128, BL], F32, tag="nrm")
                    nc.vector.tensor_tensor(nrm, cen, bcst[:, BL:2 * BL],
                                            op=ALU.mult)
                    nc.scalar.activation(
                        out=stack[fc][:, :, i - 1], in_=nrm, func=AF.Gelu,
                        bias=bes[i][:, fc:fc + 1], scale=gsc[i][:, fc:fc + 1])

            for name, i in [("s5", 5), ("s4", 4), ("s3", 3), ("s2", 2), ("s1", 1)]:
                pt = pool_stage(name)
                project(i, pt)

            # ---------- mixture ----------
            mix = hbuf.tile([128, 4, BL], F32, tag="mix")
            for fc in range(4):
                prod = lnp.tile([128, BL, 5], F32, tag="prod")
                nc.vector.tensor_tensor(prod, stack[fc], bcast_m, op=ALU.mult)
                nc.vector.tensor_reduce(out=mix[:, fc, :], in_=prod, axis=AX.X,
                                        op=ALU.add)

            # ---------- classifier ----------
            c1 = hbuf.tile([128, 8, BL], F32, tag="c1")
            dense(wc1, 4, lambda kc: mix[:, kc, :], 8, bc1, "relu", c1)
            c2 = hbuf.tile([128, 4, BL], F32, tag="c2")
            dense(wc2, 8, lambda kc: c1[:, kc, :], 4, bc2, "relu", c2)

            logits = stage.tile([BL, C], F32, tag="logits")
            for fc in range(8):
                ps = psmm.tile([125, BL], F32, tag="mm")
                for kc in range(4):
                    nc.tensor.matmul(ps, wc3[:, kc, fc * 125:(fc + 1) * 125],
                                     c2[:, kc, :],
                                     start=(kc == 0), stop=(kc == 3))
                lt = lnp.tile([125, BL], F32, tag="lt")
                nc.vector.tensor_scalar_add(lt, ps, bc3[:, fc:fc + 1])
                ps_t = pstp.tile([128, 128], F32, tag="tp")
                nc.tensor.transpose(ps_t[0:BL, 0:125], lt, ident[0:125, 0:125])
                nc.scalar.copy(logits[:, fc * 125:(fc + 1) * 125],
                               ps_t[0:BL, 0:125])
            nc.sync.dma_start(out=out_logits[:], in_=logits)

    nc.compile()
    return nc


_NC = None


def kernel(**inputs):
    global _NC, LAST_RESULTS
    if _NC is None:
        _NC = _build()

    per_core_names = list(STAGES.keys())
    in_maps = []
    for core in range(NCORES):
        sl = slice(core * BL, (core + 1) * BL)
        im = {}
        for k, v in inputs.items():
            a = np.ascontiguousarray(np.asarray(v), dtype=np.float32)
            im[k] = a[sl] if k in per_core_names else a
        in_maps.append(im)

    import os
    res = run_bass_kernel_spmd(
        _NC, in_maps, core_ids=list(range(NCORES)),
        trace=bool(os.environ.get("KERNEL_TRACE")))
    LAST_RESULTS = res

    logits = np.concatenate([res.results[i]["out_logits"] for i in range(NCORES)])
    gate = np.concatenate([res.results[i]["out_gate"] for i in range(NCORES)])
    topi = np.concatenate([res.results[i]["out_topi"] for i in range(NCORES)])
    w = np.concatenate([res.results[i]["out_w"] for i in range(NCORES)])
    return logits, gate, topi.astype(np.int32), w
